# revision 1
# baseline (speedup 1.0000x reference)
"""Trainium2 Bass kernel for nn_ExcEmbedding (ragged caption/image cosine sims).

Final design (baseline ~114us -> ~80-86us measured; run-to-run variance of
~15us comes from one straggler core's launch skew, which every
barrier-containing kernel inherits):
  - fp8e4 row streams (host converts), bf16 indicators/weights/matmuls,
    f32 PSUM + epilogues. rel fro error ~1.4e-4.
  - All input DMAs enqueued up front; xin holds all 25 row tiles.
  - s1/s2 stats packed in one [64, 1024] PSUM tile per phase (rows 0-31 sum
    of y, rows 32-63 sum of y^2) -> 2 banks per phase.
  - ACT stays in the sqrt table set the whole kernel (dummy Sqrt preloads
    it; Prelu/Square/Relu are in every set); one switch to the sigmoid set
    at the gate. rsqrt = reciprocal_approx_fast(ACT Sqrt) + one
    Newton-Raphson pass (the sqrt table alone is only ~0.4% accurate).
  - Both criticals contain ONLY Pool-engine instructions, so no compute
    engine stalls at the inter-core barrier: crit A (sem clears + broadcast
    descgen + kernel barrier + trigger) runs behind the img phase; crit B
    (wait rsem>=16 + a memset of a vv padding column) hands the finals a
    RAW dep on the landed remote data.
  - Cap-phase elementwise ops take a bias/scalar AP produced from my_vb, so
    the Tile scheduler cannot hoist them ahead of crit A's entry snapshot
    (that would delay every core's barrier arrival by ~15us). A tiny WAW
    copy likewise chains the landing-dependent vt2 square behind the gate.
  - num and vg share one [64, B] PSUM accumulator via a packed [at|gt]
    lhsT, so the gathered V^T streams through the PE once for both.
  - The final normalization epilogue runs on the HOST in f64: the device
    ships num/vg [64,256], q2 [32,256], cv [32,1024] per core.
"""

import os
import numpy as np
import ml_dtypes

import concourse.bass as bass
import concourse.bacc as bacc
import concourse.mybir as mybir
import concourse.tile as tile
from concourse.bass_utils import run_bass_kernel_spmd

F32 = mybir.dt.float32
BF16 = mybir.dt.bfloat16
AF = mybir.ActivationFunctionType
ALU = mybir.AluOpType

NCORES = 8
B = 256
R = 36
T = 64
D = 1024
DSQ = 128
M = B // NCORES          # 32 local captions / images per core
NI = M * R // 128        # 9 img row tiles of (128, D)
NC = M * T // 128        # 16 cap row tiles of (128, D)
KD = D // 128            # 8 d-blocks
SEG = KD * M             # 256 columns per (rank, stat) block
SEG2 = 2 * SEG           # vt + vt2 per rank
CAP_PRE_TP = 3           # cap tiles issued before the v transposes

NOCOLL = os.environ.get("KV2_NOCOLL", "0") == "1"
FP8_IN = os.environ.get("KV2_FP8", "1") == "1"
IN_DT_NP = "float8_e4m3" if FP8_IN else "bfloat16"


def leaky_on_act(g):
    return g % 5 != 4


def square_on_dve(g):
    return True


def build_program(beta: float):
    nc = bacc.Bacc("TRN2", target_bir_lowering=False, debug=False,
                   num_devices=NCORES)

    IN_DT = mybir.dt.float8e4 if FP8_IN else BF16
    img_rows = nc.dram_tensor("img_rows", [NI * 128, D], IN_DT, kind="ExternalInput")
    cap_rows = nc.dram_tensor("cap_rows", [NC * 128, D], IN_DT, kind="ExternalInput")
    ei_t = nc.dram_tensor("ei_t", [128, NI * M], BF16, kind="ExternalInput")
    ec2_t = nc.dram_tensor("ec2_t", [128, NC * 2 * M], BF16, kind="ExternalInput")
    w_sq_t = nc.dram_tensor("w_sq_t", [128, D], BF16, kind="ExternalInput")
    w_ex_t = nc.dram_tensor("w_ex_t", [128, D], BF16, kind="ExternalInput")
    b_sq_t = nc.dram_tensor("b_sq_t", [DSQ, 1], F32, kind="ExternalInput")
    bexp_full = nc.dram_tensor("bexp_full", [128, SEG], F32, kind="ExternalInput")
    rlens = nc.dram_tensor("rlens", [M, 1], F32, kind="ExternalInput")
    idn32 = nc.dram_tensor("idn32", [M, M], F32, kind="ExternalInput")
    nvg_out = nc.dram_tensor("nvg_out", [2 * M, B], F32, kind="ExternalOutput")
    q2_out = nc.dram_tensor("q2_out", [M, B], F32, kind="ExternalOutput")
    cv_out = nc.dram_tensor("cv_out", [M, D], F32, kind="ExternalOutput")

    rsem = nc.alloc_semaphore(name="rsem")
    lsem = nc.alloc_semaphore(name="lsem")
    psem = nc.alloc_semaphore(name="psem")

    with tile.TileContext(nc) as tc:
        with (
            tc.tile_pool(name="consts", bufs=1) as consts,
            tc.tile_pool(name="xin", bufs=NI + NC + 2) as xin,
            tc.tile_pool(name="lt", bufs=4) as lt,
            tc.tile_pool(name="yp", bufs=6) as yp,
            tc.tile_pool(name="y2p", bufs=6) as y2p,
            tc.tile_pool(name="ep", bufs=1) as ep,
            tc.tile_pool(name="smalls", bufs=1) as smalls,
            tc.tile_pool(name="tsb", bufs=1) as tsb,
            tc.tile_pool(name="psA", bufs=2, space="PSUM") as psA,
            tc.tile_pool(name="psT", bufs=1, space="PSUM") as psT,
            tc.tile_pool(name="psF", bufs=1, space="PSUM") as psF,
        ):
            # ---- all input DMAs enqueued first (sync queue order) ----
            ei_sb = consts.tile([128, NI, M], BF16)
            nc.sync.dma_start(ei_sb[:], ei_t[:].rearrange("p (t c) -> p t c", t=NI))
            idn_sb = consts.tile([M, M], F32)
            nc.sync.dma_start(idn_sb[:], idn32[:])
            xs = []
            for t in range(NI):
                x = xin.tile([128, D], IN_DT, name="x")
                nc.sync.dma_start(x[:], img_rows[128 * t:128 * (t + 1), :])
                xs.append(x)
            ec_sb = consts.tile([128, NC, 2 * M], BF16)
            nc.sync.dma_start(ec_sb[:], ec2_t[:].rearrange("p (t c) -> p t c", t=NC))
            xcs = []
            for t in range(NC):
                xc = xin.tile([128, D], IN_DT, name="x")
                nc.sync.dma_start(xc[:], cap_rows[128 * t:128 * (t + 1), :])
                xcs.append(xc)
            wsq_sb = consts.tile([128, D], BF16)
            nc.sync.dma_start(wsq_sb[:], w_sq_t[:])
            wex_sb = consts.tile([128, D], BF16)
            nc.sync.dma_start(wex_sb[:], w_ex_t[:])
            bsq_sb = consts.tile([DSQ, 1], F32)
            nc.sync.dma_start(bsq_sb[:], b_sq_t[:])
            bexp_sb = consts.tile([128, SEG], F32)
            nc.sync.dma_start(bexp_sb[:], bexp_full[:])
            rlens_sb = consts.tile([M, 1], F32)
            nc.sync.dma_start(rlens_sb[:], rlens[:])

            # ---- dummy Sqrt pins the sqrt table set during DMA warmup ----
            dumm = smalls.tile([1, 1], F32, name="dumm")
            nc.vector.memset(dumm[:], 1.0)
            dum2 = smalls.tile([1, 1], F32, name="dum2")
            nc.scalar.activation(dum2[:], dumm[:], AF.Sqrt)

            # dep_b/dep_s: [128,1] bias=0 / scalar=0.1 APs produced from
            # my_vb. Threading them through the cap-phase leakys stops the
            # scheduler from hoisting cap elementwise work ahead of critical
            # A's entry snapshot (that would delay every core's barrier
            # arrival by ~15us).
            def leaky_square(x, g, dep_b=None, dep_s=None):
                y = yp.tile([128, D], BF16, name="y")
                if leaky_on_act(g):
                    nc.scalar.activation(y[:], x[:], AF.Prelu, alpha=0.1,
                                         bias=dep_b[:] if dep_b is not None
                                         else 0.0)
                else:
                    nc.vector.scalar_tensor_tensor(
                        y[:], x[:], dep_s[:] if dep_s is not None else 0.1,
                        x[:], op0=ALU.mult, op1=ALU.max)
                y2 = y2p.tile([128, D], BF16, name="y2")
                if square_on_dve(g):
                    nc.vector.tensor_tensor(y2[:], y[:], y[:], op=ALU.mult)
                else:
                    nc.scalar.square(y2[:], y[:])
                return y, y2

            # ---- img phase ----
            s12i = psA.tile([2 * M, D], F32, tag="acc", name="s12i")
            for t in range(NI):
                y, y2 = leaky_square(xs[t], t)
                for h in range(2):
                    cs = slice(512 * h, 512 * (h + 1))
                    nc.tensor.matmul(s12i[0:M, cs], ei_sb[:, t, :], y[:, cs],
                                     start=(t == 0), stop=(t == NI - 1),
                                     skip_group_check=True)
                    nc.tensor.matmul(s12i[M:2 * M, cs], ei_sb[:, t, :], y2[:, cs],
                                     start=(t == 0), stop=(t == NI - 1),
                                     skip_group_check=True)

            # rsqrt with one Newton-Raphson pass: the ACT sqrt table is only
            # ~0.4% accurate (65536 ULP budget); r1 = r0*(1.5 - 0.5*x*r0^2)
            # squares that error away. Runs per 512-col half so transposes
            # can start after the first half.
            def rsqrt_half(s12_tile, cs, nm):
                sq = ep.tile([M, 512], F32, name=f"sq{nm}")
                nc.scalar.activation(sq[:], s12_tile[M:2 * M, cs], AF.Sqrt)
                r0 = ep.tile([M, 512], F32, name=f"r0{nm}")
                nc.vector.reciprocal_approx_fast(r0[:], sq[:])
                a = ep.tile([M, 512], F32, name=f"a{nm}")
                nc.vector.tensor_tensor(a[:], r0[:], r0[:], op=ALU.mult)
                b = ep.tile([M, 512], F32, name=f"b{nm}")
                nc.vector.scalar_tensor_tensor(b[:], s12_tile[M:2 * M, cs],
                                               -0.5, a[:],
                                               op0=ALU.mult, op1=ALU.mult)
                c = ep.tile([M, 512], F32, name=f"c{nm}")
                nc.vector.tensor_scalar_add(c[:], b[:], 1.5)
                r1 = ep.tile([M, 512], F32, name=f"r1{nm}")
                nc.vector.tensor_tensor(r1[:], r0[:], c[:], op=ALU.mult)
                return r1

            # ---- cap phase part 1 (keeps the PE busy during img epilogue) ----
            s12c = psA.tile([2 * M, D], F32, tag="acc", name="s12c")

            def cap_tile(t):
                yc, yc2 = leaky_square(xcs[t], NI + t, dep_b=zb, dep_s=pt1)
                for h in range(2):
                    cs = slice(512 * h, 512 * (h + 1))
                    nc.tensor.matmul(s12c[0:M, cs], ec_sb[:, t, 0:M], yc[:, cs],
                                     start=(t == 0), stop=(t == NC - 1),
                                     skip_group_check=True)
                    nc.tensor.matmul(s12c[M:2 * M, cs], ec_sb[:, t, M:2 * M],
                                     yc2[:, cs],
                                     start=(t == 0), stop=(t == NC - 1),
                                     skip_group_check=True)

            # ---- img epilogue (by halves) + transpose v -> my_vb bf16 ----
            v = smalls.tile([M, D], F32, name="v")
            vps = psT.tile([128, SEG], F32, tag="t", name="vps")
            my_vb = tsb.tile([128, SEG], BF16, name="my_vb")
            cap_issued = 0
            for h in range(2):
                cs = slice(512 * h, 512 * (h + 1))
                r1 = rsqrt_half(s12i, cs, f"i{h}")
                nc.vector.scalar_tensor_tensor(v[:, cs], s12i[0:M, cs],
                                               1.0 / R, r1[:],
                                               op0=ALU.mult, op1=ALU.mult)
                for k in range(4 * h, 4 * (h + 1)):
                    nc.tensor.transpose(vps[:, M * k:M * (k + 1)],
                                        v[:, 128 * k:128 * (k + 1)], idn_sb[:])
                nc.vector.tensor_copy(my_vb[:, 128 * h:128 * (h + 1)],
                                      vps[:, 128 * h:128 * (h + 1)])

            zb = smalls.tile([128, 1], F32, name="zb")
            nc.vector.tensor_scalar_mul(zb[:], my_vb[:, 0:1], 0.0)
            pt1 = smalls.tile([128, 1], F32, name="pt1")
            nc.vector.tensor_scalar_add(pt1[:], zb[:], 0.1)

            # ---- critical A: Pool-only, so no other engine stalls on the
            # inter-core barrier; it orders clears before any flight ----
            vv = tsb.tile([128, NCORES * SEG + 2], BF16, name="vv")
            if NOCOLL:
                for g in range(NCORES):
                    nc.vector.tensor_copy(vv[:, SEG * g:SEG * (g + 1)],
                                          my_vb[:])
            else:
                with tc.tile_critical(no_gpsimd_drain=True):
                    nc.gpsimd.sem_clear(rsem)
                    nc.gpsimd.sem_clear(lsem)
                    nc.gpsimd.sem_clear(psem)
                    rank = nc.gpsimd.partition_id()
                    nc.gpsimd.remote_dma_broadcast(
                        vv[:, bass.ds(rank * SEG, SEG)], my_vb[:],
                        remote_sem=rsem, local_sem=lsem,
                        rdests=[(0, j) for j in range(NCORES)],
                    ).then_inc(psem, 1)
                    nc.gpsimd.wait_ge(psem, 1)
                    nc.gpsimd.bir_kernel_barrier_wait([list(range(NCORES))])
                    nc.gpsimd.trigger_dma(count=1)

            # ---- cap phase part 2 ----
            for t in range(cap_issued, NC):
                cap_tile(t)

            # ---- cap epilogue (by halves) + transpose cv -> cvt bf16 ----
            cv = smalls.tile([M, D], F32, name="cv")
            cvps = psT.tile([128, SEG], F32, tag="t2", name="cvps")
            cvt = tsb.tile([128, SEG], BF16, name="cvt")
            for h in range(2):
                cs = slice(512 * h, 512 * (h + 1))
                r1 = rsqrt_half(s12c, cs, f"c{h}")
                nc.vector.scalar_tensor_tensor(cv[:, cs], s12c[0:M, cs],
                                               rlens_sb[:], r1[:],
                                               op0=ALU.mult, op1=ALU.mult)
                for k in range(4 * h, 4 * (h + 1)):
                    nc.tensor.transpose(cvps[:, M * k:M * (k + 1)],
                                        cv[:, 128 * k:128 * (k + 1)], idn_sb[:])
                nc.vector.tensor_copy(cvt[:, 128 * h:128 * (h + 1)],
                                      cvps[:, 128 * h:128 * (h + 1)])
            nc.sync.dma_start(cv_out[:], cv[:])

            # ---- gate ----
            ht_ps = psF.tile([DSQ, M], F32, tag="f", name="ht_ps")
            for k in range(KD):
                nc.tensor.matmul(ht_ps[:], wsq_sb[:, 128 * k:128 * (k + 1)],
                                 cvt[:, M * k:M * (k + 1)],
                                 start=(k == 0), stop=(k == KD - 1),
                                 skip_group_check=True)
            ht = tsb.tile([DSQ, M], BF16, name="ht")
            nc.scalar.activation(ht[:], ht_ps[:], AF.Relu, bias=bsq_sb[:])

            gps = psT.tile([128, SEG], F32, tag="g", name="gps")
            for k in range(KD):
                nc.tensor.matmul(gps[:, M * k:M * (k + 1)],
                                 wex_sb[:, 128 * k:128 * (k + 1)], ht[:],
                                 skip_group_check=True)
            gpb = tsb.tile([128, SEG], F32, name="gpb")
            nc.vector.tensor_tensor(gpb[:], gps[:], bexp_sb[:], op=ALU.add)
            gt = tsb.tile([128, SEG], BF16, name="gt")
            nc.scalar.activation(gt[:], gpb[:], AF.Sigmoid)
            g2t = tsb.tile([128, SEG], BF16, name="g2t")
            nc.vector.tensor_tensor(g2t[:], gt[:], gt[:], op=ALU.mult)
            # pack [at | gt] per k-block so num and vg share one matmul each:
            # agt[:, 64k:64k+32] = at_k, agt[:, 64k+32:64k+64] = gt_k
            agt = tsb.tile([128, 2 * SEG], BF16, name="agt")
            for k in range(KD):
                ks = slice(M * k, M * (k + 1))
                nc.vector.tensor_tensor(agt[:, 2 * M * k:2 * M * k + M],
                                        gt[:, ks], cvt[:, ks], op=ALU.mult)
                nc.vector.tensor_copy(agt[:, 2 * M * k + M:2 * M * (k + 1)],
                                      gt[:, ks])

            # ---- critical B: wait for the gathered payload; the memset of a
            # padding column of vv gives the finals a RAW dep on it ----
            if not NOCOLL:
                with tc.tile_critical(no_gpsimd_drain=True):
                    nc.gpsimd.wait_ge(rsem, NCORES * 2)
                    nc.gpsimd.memset(vv[0:1, NCORES * SEG:NCORES * SEG + 2], 0)

            # ---- finals: [num|vg] packed [2M, B], q2 [M, B] ----
            # The tiny copy chains vt2 behind agt (WAW on vt2[0,0]) so the
            # scheduler cannot hoist the landing-dependent vt2 square ahead
            # of the cap epilogue / gate chain in the DVE queue.
            vt2 = tsb.tile([128, NCORES * SEG], BF16, name="vt2")
            nc.vector.tensor_copy(vt2[0:1, 0:2], agt[0:1, 0:2])
            nc.vector.tensor_tensor(vt2[:], vv[:, 0:NCORES * SEG],
                                    vv[:, 0:NCORES * SEG], op=ALU.mult)
            vv4 = vv[:, 0:NCORES * SEG].rearrange("p (g k c) -> p g k c",
                                                  g=NCORES, k=KD)
            vt24 = vt2[:].rearrange("p (g k c) -> p g k c", g=NCORES, k=KD)
            nvg_ps = psF.tile([2 * M, 512], F32, tag="f", name="nvg_ps")
            q2_ps = psF.tile([M, 512], F32, tag="f", name="q2_ps")
            for k in range(KD):
                nc.tensor.matmul(nvg_ps[:, 0:B], agt[:, 2 * M * k:2 * M * (k + 1)],
                                 vv4[:, :, k, :],
                                 start=(k == 0), stop=(k == KD - 1),
                                 skip_group_check=True)
            nvgs = smalls.tile([2 * M, B], F32, name="nvgs")
            nc.vector.tensor_copy(nvgs[:], nvg_ps[:, 0:B])
            nc.sync.dma_start(nvg_out[:], nvgs[:])
            for k in range(KD):
                nc.tensor.matmul(q2_ps[:, 0:B], g2t[:, M * k:M * (k + 1)],
                                 vt24[:, :, k, :],
                                 start=(k == 0), stop=(k == KD - 1),
                                 skip_group_check=True)
            qsb = smalls.tile([M, B], F32, name="qsb")
            nc.vector.tensor_copy(qsb[:], q2_ps[:, 0:B])
            nc.sync.dma_start(q2_out[:], qsb[:])

    nc.compile()
    return nc


_PROG_CACHE: dict = {}


def get_program(beta: float):
    if beta not in _PROG_CACHE:
        _PROG_CACHE[beta] = build_program(beta)
    return _PROG_CACHE[beta]


def make_in_maps(img_embed, cap_embed, lens, W_sq, b_sq, W_ex, b_ex):
    bf = ml_dtypes.bfloat16
    in_dt = getattr(ml_dtypes, IN_DT_NP)
    img_bf = np.ascontiguousarray(img_embed, dtype=np.float32).astype(in_dt)
    cap_bf = np.ascontiguousarray(cap_embed, dtype=np.float32).astype(in_dt)
    lens_i = np.asarray(lens).astype(np.int64)

    # W_sq (D, DSQ) -> [128, KD*128]: w_sq_t[p, 128k+j] = W_sq[128k+p, j]
    w_sq_np = np.asarray(W_sq, dtype=np.float32).astype(bf)
    w_sq_t_np = np.ascontiguousarray(
        w_sq_np.reshape(KD, 128, DSQ).transpose(1, 0, 2).reshape(128, D))
    w_ex_t_np = np.ascontiguousarray(np.asarray(W_ex, dtype=np.float32).astype(bf))
    b_sq_np = np.ascontiguousarray(
        np.asarray(b_sq, dtype=np.float32).reshape(DSQ, 1))
    # bexp_full[p, M*k + c] = +b_ex[128k + p]
    bex = np.asarray(b_ex, dtype=np.float32)
    bexp_np = np.ascontiguousarray(
        np.repeat(bex.reshape(KD, 128).T, M, axis=1).reshape(128, SEG))
    idn_np = np.eye(M, dtype=np.float32)

    ei_np = np.zeros((NI * 128, M), dtype=np.float32)
    rows_i = np.arange(M * R)
    ei_np[rows_i, rows_i // R] = 1.0
    ei_t_np = ei_np.reshape(NI, 128, M).transpose(1, 0, 2).reshape(
        128, NI * M).astype(bf)

    in_maps = []
    for j in range(NCORES):
        sl = slice(M * j, M * (j + 1))
        lens_local = lens_i[sl]
        ec2_np = np.zeros((M * T, 2 * M), dtype=np.float32)
        rows = np.arange(M * T)
        cidx = rows // T
        tidx = rows % T
        ec2_np[rows, M + cidx] = 1.0
        keep = tidx < lens_local[cidx]
        ec2_np[rows[keep], cidx[keep]] = 1.0
        ec2_t_np = ec2_np.reshape(NC, 128, 2 * M).transpose(1, 0, 2).reshape(
            128, NC * 2 * M).astype(bf)
        rlens_np = (1.0 / lens_local.astype(np.float64)).astype(
            np.float32).reshape(M, 1)

        in_maps.append({
            "img_rows": np.ascontiguousarray(img_bf[sl].reshape(M * R, D)),
            "cap_rows": np.ascontiguousarray(cap_bf[sl].reshape(M * T, D)),
            "ei_t": np.ascontiguousarray(ei_t_np),
            "ec2_t": np.ascontiguousarray(ec2_t_np),
            "w_sq_t": w_sq_t_np,
            "w_ex_t": w_ex_t_np,
            "b_sq_t": b_sq_np,
            "bexp_full": bexp_np,
            "rlens": rlens_np,
            "idn32": idn_np,
        })
    return in_maps


LAST_RESULT = None
EPS = 1e-8


def kernel(img_embed, cap_embed, lens, W_sq, b_sq, W_ex, b_ex, beta, beta1):
    global LAST_RESULT
    beta_f = float(np.asarray(beta).reshape(-1)[0])
    nc = get_program(beta_f)
    in_maps = make_in_maps(img_embed, cap_embed, lens, W_sq, b_sq, W_ex, b_ex)
    res = run_bass_kernel_spmd(nc, in_maps, core_ids=list(range(NCORES)))
    LAST_RESULT = res
    sims = np.empty((B, B), dtype=np.float32)
    for j in range(NCORES):
        r = res.results[j]
        nvg = r["nvg_out"].astype(np.float64)   # (2M, B)
        num = nvg[0:M]
        vg = nvg[M:2 * M]
        q2 = r["q2_out"].astype(np.float64)
        cv = r["cv_out"].astype(np.float64)     # (M, D)
        rn = 1.0 / (np.sqrt((cv * cv).sum(axis=1, keepdims=True)) + EPS)
        bias = beta_f * cv.sum(axis=1, keepdims=True) * rn
        denom = np.sqrt(q2 + 2.0 * beta_f * vg + beta_f * beta_f * D) + EPS
        simst = (num * rn + bias) / denom       # (M, B) = sims[:, block].T
        sims[:, M * j:M * (j + 1)] = simst.T.astype(np.float32)
    return sims



# revision 12
# speedup vs baseline: 1.3857x; 1.3857x over previous
"""Trainium2 Bass kernel for nn_ExcEmbedding (ragged caption/image cosine sims).

v2 design (baseline v1 ~90-92us):
  - fp8e4 row streams (host converts), bf16 indicators/weights/matmuls,
    f32 PSUM + epilogues. Host epilogue in f64 does the final normalization.
  - Batched input DMAs: img rows in 3 chunks, cap rows in 4 chunks, one
    DMA per aux tensor. Big DMAs split across all 16 SDMA engines.
  - rsqrt via the ACT Rsqrt table (reciprocal_sqrt_and_small set, which
    also holds parametric_relu/relu/square/copy). No Newton-Raphson: the
    ~0.4% table error is far inside the 2e-2 rel-err budget.
  - No mid-kernel inter-core barrier and no semaphore clears: user sems
    are zeroed by the previous run's postamble (and at NEFF init), and a
    remote payload can only arrive ~25us+ after its sender started, long
    past the receiver's preamble. The broadcast descriptor prep sits at
    the top of the Pool queue (pays the ~6us Q7 lib load during the input
    DMAs); a Pool-only critical after the img epilogue just waits for the
    prep sem and fires trigger_dma, so each core's payload flies as soon
    as ITS my_vb is ready instead of after a global barrier.
  - Cap-phase elementwise ops take a bias/scalar AP produced from my_vb
    so the Tile scheduler cannot hoist them ahead of the trigger
    critical's entry snapshot (that would delay the broadcast).
  - num and vg share one [64, B] PSUM accumulator via a packed [at|gt]
    lhsT; q2 uses vt2 = vv*vv. Finals run after crit B (wait rsem>=16).
"""

import os
import numpy as np
import ml_dtypes

import concourse.bass as bass
import concourse.bacc as bacc
import concourse.mybir as mybir
import concourse.tile as tile
from concourse.bass_utils import run_bass_kernel_spmd

F32 = mybir.dt.float32
BF16 = mybir.dt.bfloat16
AF = mybir.ActivationFunctionType
ALU = mybir.AluOpType

NCORES = 8
B = 256
R = 36
T = 64
D = 1024
DSQ = 128
M = B // NCORES          # 32 local captions / images per core
NI = M * R // 128        # 9 img row tiles of (128, D)
NC = M * T // 128        # 16 cap row tiles of (128, D)
KD = D // 128            # 8 d-blocks
SEG = KD * M             # 256 columns per (rank, stat) block
IMG_CHUNK = 3            # img tiles per input DMA
CAP_CHUNK = 4            # cap tiles per input DMA

NOCOLL = os.environ.get("KV2_NOCOLL", "0") == "1"
FP8_IN = os.environ.get("KV2_FP8", "1") == "1"
IN_DT_NP = "float8_e4m3" if FP8_IN else "bfloat16"


def leaky_on_act(g):
    return g % 5 != 4


def build_program(beta: float):
    nc = bacc.Bacc("TRN2", target_bir_lowering=False, debug=False,
                   num_devices=NCORES)

    IN_DT = mybir.dt.float8e4 if FP8_IN else BF16
    img_rows = nc.dram_tensor("img_rows", [NI * 128, D], IN_DT, kind="ExternalInput")
    cap_rows = nc.dram_tensor("cap_rows", [NC * 128, D], IN_DT, kind="ExternalInput")
    ei_t = nc.dram_tensor("ei_t", [128, NI * M], BF16, kind="ExternalInput")
    ec2_t = nc.dram_tensor("ec2_t", [128, NC * 2 * M], BF16, kind="ExternalInput")
    w_sq_t = nc.dram_tensor("w_sq_t", [128, D], BF16, kind="ExternalInput")
    w_ex_t = nc.dram_tensor("w_ex_t", [128, D], BF16, kind="ExternalInput")
    b_sq_t = nc.dram_tensor("b_sq_t", [DSQ, 1], F32, kind="ExternalInput")
    bexp_full = nc.dram_tensor("bexp_full", [128, SEG], F32, kind="ExternalInput")
    rlens = nc.dram_tensor("rlens", [M, 1], F32, kind="ExternalInput")
    idn32 = nc.dram_tensor("idn32", [M, M], F32, kind="ExternalInput")
    nvg_out = nc.dram_tensor("nvg_out", [2 * M, B], F32, kind="ExternalOutput")
    q2_out = nc.dram_tensor("q2_out", [M, B], F32, kind="ExternalOutput")
    cv_out = nc.dram_tensor("cv_out", [M, D], F32, kind="ExternalOutput")

    rsem = nc.alloc_semaphore(name="rsem")
    lsem = nc.alloc_semaphore(name="lsem")
    psem = nc.alloc_semaphore(name="psem")

    with tile.TileContext(nc) as tc:
        with (
            tc.tile_pool(name="consts", bufs=1) as consts,
            tc.tile_pool(name="xin", bufs=1) as xin,
            tc.tile_pool(name="yp", bufs=6) as yp,
            tc.tile_pool(name="y2p", bufs=6) as y2p,
            tc.tile_pool(name="ep", bufs=1) as ep,
            tc.tile_pool(name="smalls", bufs=1) as smalls,
            tc.tile_pool(name="tsb", bufs=1) as tsb,
            tc.tile_pool(name="psA", bufs=2, space="PSUM") as psA,
            tc.tile_pool(name="psT", bufs=1, space="PSUM") as psT,
            tc.tile_pool(name="psF", bufs=1, space="PSUM") as psF,
        ):
            # ---- input DMAs, batched; small tensors needed early first ----
            ei_sb = consts.tile([128, NI, M], BF16)
            nc.sync.dma_start(ei_sb[:], ei_t[:].rearrange("p (t c) -> p t c", t=NI))
            idn_sb = consts.tile([M, M], F32)
            nc.sync.dma_start(idn_sb[:], idn32[:])
            rlens_sb = consts.tile([M, 1], F32)
            nc.sync.dma_start(rlens_sb[:], rlens[:])
            bsq_sb = consts.tile([DSQ, 1], F32)
            nc.sync.dma_start(bsq_sb[:], b_sq_t[:])

            ximg = xin.tile([128, NI, D], IN_DT, name="ximg")
            for j in range(0, NI, IMG_CHUNK):
                e = min(j + IMG_CHUNK, NI)
                nc.sync.dma_start(
                    ximg[:, j:e, :],
                    img_rows[128 * j:128 * e, :].rearrange(
                        "(t p) d -> p t d", p=128))
            xcap = xin.tile([128, NC, D], IN_DT, name="xcap")
            for j in range(0, NC, CAP_CHUNK):
                e = min(j + CAP_CHUNK, NC)
                nc.sync.dma_start(
                    xcap[:, j:e, :],
                    cap_rows[128 * j:128 * e, :].rearrange(
                        "(t p) d -> p t d", p=128))

            ec_sb = consts.tile([128, NC, 2 * M], BF16)
            nc.sync.dma_start(ec_sb[:], ec2_t[:].rearrange("p (t c) -> p t c", t=NC))
            wsq_sb = consts.tile([128, D], BF16)
            nc.sync.dma_start(wsq_sb[:], w_sq_t[:])
            wex_sb = consts.tile([128, D], BF16)
            nc.sync.dma_start(wex_sb[:], w_ex_t[:])
            bexp_sb = consts.tile([128, SEG], F32)
            nc.sync.dma_start(bexp_sb[:], bexp_full[:])

            # ---- dummy Sqrt pins the sqrt table set early ----
            dumm = smalls.tile([1, 1], F32, name="dumm")
            nc.vector.memset(dumm[:], 1.0)
            dum2 = smalls.tile([1, 1], F32, name="dum2")
            nc.scalar.activation(dum2[:], dumm[:], AF.Sqrt)

            # ---- broadcast descriptor prep at the top of the Pool queue.
            # Pays the ~6us Q7 lib load during the input DMAs. The data read
            # of my_vb is deferred to trigger time; the trigger critical's
            # entry snapshot (after the img epilogue) provides the my_vb
            # write ordering. ----
            vv = tsb.tile([128, NCORES * SEG + 2], BF16, name="vv")
            my_vb = tsb.tile([128, SEG], BF16, name="my_vb")
            if not NOCOLL:
                # Register the prelude AllGather (NRT comm init / collectives
                # setup) without emitting a barrier wait anywhere — the
                # remote broadcast path needs the comm initialized, but no
                # core has to block on the (slow, ~70us) 1-byte AllGather.
                nc._bir_kernel_barrier_sem_replica_groups.append(
                    set(range(NCORES)))
                # Early Pool-only critical: compute the rank and pay the ~6us
                # Q7 broadcast-lib IRAM load now, while the input DMAs
                # stream, so the descgen in the trigger critical is cheap.
                from concourse import library_config
                with tc.tile_critical(no_gpsimd_drain=True):
                    rank = nc.gpsimd.partition_id()
                    nc.gpsimd.load_library(library_config.remote_dma)

            # dep_b/dep_s: [128,1] bias=0 / scalar=0.1 APs produced from
            # my_vb. Threading them through the cap-phase elementwise ops
            # stops the scheduler from hoisting cap work ahead of the
            # trigger critical's entry snapshot.
            def leaky_square(x, g, dep_b=None, dep_s=None):
                y = yp.tile([128, D], BF16, name="y")
                if leaky_on_act(g):
                    nc.scalar.activation(y[:], x[:], AF.Prelu, alpha=0.1,
                                         bias=dep_b[:] if dep_b is not None
                                         else 0.0)
                else:
                    nc.vector.scalar_tensor_tensor(
                        y[:], x[:], dep_s[:] if dep_s is not None else 0.1,
                        x[:], op0=ALU.mult, op1=ALU.max)
                y2 = y2p.tile([128, D], BF16, name="y2")
                nc.vector.tensor_tensor(y2[:], y[:], y[:], op=ALU.mult)
                return y, y2

            # ---- img phase ----
            s12i = psA.tile([2 * M, D], F32, tag="acc", name="s12i")
            for t in range(NI):
                y, y2 = leaky_square(ximg[:, t, :], t)
                for h in range(2):
                    cs = slice(512 * h, 512 * (h + 1))
                    nc.tensor.matmul(s12i[0:M, cs], ei_sb[:, t, :], y[:, cs],
                                     start=(t == 0), stop=(t == NI - 1),
                                     skip_group_check=True)
                    nc.tensor.matmul(s12i[M:2 * M, cs], ei_sb[:, t, :], y2[:, cs],
                                     start=(t == 0), stop=(t == NI - 1),
                                     skip_group_check=True)

            # ---- img epilogue (by halves) + transpose v -> my_vb bf16 ----
            v = smalls.tile([M, D], F32, name="v")
            vps = psT.tile([128, SEG], F32, tag="t", name="vps")
            for h in range(2):
                cs = slice(512 * h, 512 * (h + 1))
                sq = ep.tile([M, 512], F32, name=f"sqi{h}")
                nc.scalar.activation(sq[:], s12i[M:2 * M, cs], AF.Sqrt)
                r1 = ep.tile([M, 512], F32, name=f"ri{h}")
                nc.vector.reciprocal_approx_fast(r1[:], sq[:])
                nc.vector.scalar_tensor_tensor(v[:, cs], s12i[0:M, cs],
                                               1.0 / R, r1[:],
                                               op0=ALU.mult, op1=ALU.mult)
                for k in range(4 * h, 4 * (h + 1)):
                    nc.tensor.transpose(vps[:, M * k:M * (k + 1)],
                                        v[:, 128 * k:128 * (k + 1)], idn_sb[:])
                nc.vector.tensor_copy(my_vb[:, 128 * h:128 * (h + 1)],
                                      vps[:, 128 * h:128 * (h + 1)])

            zb = smalls.tile([128, 1], F32, name="zb")
            nc.vector.tensor_scalar_mul(zb[:], my_vb[:, 0:1], 0.0)
            pt1 = smalls.tile([128, 1], F32, name="pt1")
            nc.vector.tensor_scalar_add(pt1[:], zb[:], 0.1)

            # ---- trigger critical: Pool-only; the descgen's my_vb read puts
            # the img-epilogue dependency on this critical's entry snapshot.
            # Fires this core's broadcast immediately; no inter-core barrier.
            if NOCOLL:
                for g in range(NCORES):
                    nc.vector.tensor_copy(vv[:, SEG * g:SEG * (g + 1)],
                                          my_vb[:])
            else:
                with tc.tile_critical(no_gpsimd_drain=True):
                    nc.gpsimd.remote_dma_broadcast(
                        vv[:, bass.ds(rank * SEG, SEG)], my_vb[:],
                        remote_sem=rsem, local_sem=lsem,
                        rdests=[(0, j) for j in range(NCORES)],
                    ).then_inc(psem, 1)
                    nc.gpsimd.wait_ge(psem, 1)
                    nc.gpsimd.trigger_dma(count=1)

            # ---- cap phase ----
            s12c = psA.tile([2 * M, D], F32, tag="acc", name="s12c")
            for t in range(NC):
                yc, yc2 = leaky_square(xcap[:, t, :], NI + t, dep_b=zb,
                                       dep_s=pt1)
                for h in range(2):
                    cs = slice(512 * h, 512 * (h + 1))
                    nc.tensor.matmul(s12c[0:M, cs], ec_sb[:, t, 0:M], yc[:, cs],
                                     start=(t == 0), stop=(t == NC - 1),
                                     skip_group_check=True)
                    nc.tensor.matmul(s12c[M:2 * M, cs], ec_sb[:, t, M:2 * M],
                                     yc2[:, cs],
                                     start=(t == 0), stop=(t == NC - 1),
                                     skip_group_check=True)

            # ---- cap epilogue (by halves) + transpose cv -> cvt bf16 ----
            cv = smalls.tile([M, D], F32, name="cv")
            cvps = psT.tile([128, SEG], F32, tag="t2", name="cvps")
            cvt = tsb.tile([128, SEG], BF16, name="cvt")
            for h in range(2):
                cs = slice(512 * h, 512 * (h + 1))
                sq = ep.tile([M, 512], F32, name=f"sqc{h}")
                nc.scalar.activation(sq[:], s12c[M:2 * M, cs], AF.Sqrt)
                r1 = ep.tile([M, 512], F32, name=f"rc{h}")
                nc.vector.reciprocal_approx_fast(r1[:], sq[:])
                nc.vector.scalar_tensor_tensor(cv[:, cs], s12c[0:M, cs],
                                               rlens_sb[:], r1[:],
                                               op0=ALU.mult, op1=ALU.mult)
                for k in range(4 * h, 4 * (h + 1)):
                    nc.tensor.transpose(cvps[:, M * k:M * (k + 1)],
                                        cv[:, 128 * k:128 * (k + 1)], idn_sb[:])
                nc.vector.tensor_copy(cvt[:, 128 * h:128 * (h + 1)],
                                      cvps[:, 128 * h:128 * (h + 1)])
            nc.sync.dma_start(cv_out[:], cv[:])

            # ---- gate ----
            ht_ps = psF.tile([DSQ, M], F32, tag="f", name="ht_ps")
            for k in range(KD):
                nc.tensor.matmul(ht_ps[:], wsq_sb[:, 128 * k:128 * (k + 1)],
                                 cvt[:, M * k:M * (k + 1)],
                                 start=(k == 0), stop=(k == KD - 1),
                                 skip_group_check=True)
            ht = tsb.tile([DSQ, M], BF16, name="ht")
            nc.scalar.activation(ht[:], ht_ps[:], AF.Relu, bias=bsq_sb[:])

            gps = psT.tile([128, SEG], F32, tag="g", name="gps")
            for k in range(KD):
                nc.tensor.matmul(gps[:, M * k:M * (k + 1)],
                                 wex_sb[:, 128 * k:128 * (k + 1)], ht[:],
                                 skip_group_check=True)
            gpb = tsb.tile([128, SEG], F32, name="gpb")
            nc.vector.tensor_tensor(gpb[:], gps[:], bexp_sb[:], op=ALU.add)
            gt = tsb.tile([128, SEG], BF16, name="gt")
            nc.scalar.activation(gt[:], gpb[:], AF.Sigmoid)
            g2t = tsb.tile([128, SEG], BF16, name="g2t")
            nc.vector.tensor_tensor(g2t[:], gt[:], gt[:], op=ALU.mult)
            # pack [at | gt] per k-block so num and vg share one matmul each:
            # agt[:, 64k:64k+32] = at_k, agt[:, 64k+32:64k+64] = gt_k
            agt = tsb.tile([128, 2 * SEG], BF16, name="agt")
            for k in range(KD):
                ks = slice(M * k, M * (k + 1))
                nc.vector.tensor_tensor(agt[:, 2 * M * k:2 * M * k + M],
                                        gt[:, ks], cvt[:, ks], op=ALU.mult)
                nc.vector.tensor_copy(agt[:, 2 * M * k + M:2 * M * (k + 1)],
                                      gt[:, ks])

            # ---- critical B: wait for the gathered payload; the memset of a
            # padding column of vv gives the finals a RAW dep on it ----
            if not NOCOLL:
                with tc.tile_critical(no_gpsimd_drain=True):
                    nc.gpsimd.wait_ge(rsem, NCORES * 2)
                    nc.gpsimd.memset(vv[0:1, NCORES * SEG:NCORES * SEG + 2], 0)

            # ---- finals: [num|vg] packed [2M, B], q2 [M, B] ----
            # The tiny copy chains vt2 behind agt (WAW on vt2[0,0]) so the
            # scheduler cannot hoist the landing-dependent vt2 square ahead
            # of the cap epilogue / gate chain in the DVE queue.
            vt2 = tsb.tile([128, NCORES * SEG], BF16, name="vt2")
            nc.vector.tensor_copy(vt2[0:1, 0:2], agt[0:1, 0:2])
            nc.vector.tensor_tensor(vt2[:], vv[:, 0:NCORES * SEG],
                                    vv[:, 0:NCORES * SEG], op=ALU.mult)
            vv4 = vv[:, 0:NCORES * SEG].rearrange("p (g k c) -> p g k c",
                                                  g=NCORES, k=KD)
            vt24 = vt2[:].rearrange("p (g k c) -> p g k c", g=NCORES, k=KD)
            nvg_ps = psF.tile([2 * M, 512], F32, tag="f", name="nvg_ps")
            q2_ps = psF.tile([M, 512], F32, tag="f", name="q2_ps")
            for k in range(KD):
                nc.tensor.matmul(nvg_ps[:, 0:B], agt[:, 2 * M * k:2 * M * (k + 1)],
                                 vv4[:, :, k, :],
                                 start=(k == 0), stop=(k == KD - 1),
                                 skip_group_check=True)
            nvgs = smalls.tile([2 * M, B], F32, name="nvgs")
            nc.vector.tensor_copy(nvgs[:], nvg_ps[:, 0:B])
            nc.sync.dma_start(nvg_out[:], nvgs[:])
            for k in range(KD):
                nc.tensor.matmul(q2_ps[:, 0:B], g2t[:, M * k:M * (k + 1)],
                                 vt24[:, :, k, :],
                                 start=(k == 0), stop=(k == KD - 1),
                                 skip_group_check=True)
            qsb = smalls.tile([M, B], F32, name="qsb")
            nc.vector.tensor_copy(qsb[:], q2_ps[:, 0:B])
            nc.sync.dma_start(q2_out[:], qsb[:])

    nc.compile()
    return nc


_PROG_CACHE: dict = {}


def get_program(beta: float):
    if beta not in _PROG_CACHE:
        _PROG_CACHE[beta] = build_program(beta)
    return _PROG_CACHE[beta]


def make_in_maps(img_embed, cap_embed, lens, W_sq, b_sq, W_ex, b_ex):
    bf = ml_dtypes.bfloat16
    in_dt = getattr(ml_dtypes, IN_DT_NP)
    img_bf = np.ascontiguousarray(img_embed, dtype=np.float32).astype(in_dt)
    cap_bf = np.ascontiguousarray(cap_embed, dtype=np.float32).astype(in_dt)
    lens_i = np.asarray(lens).astype(np.int64)

    # W_sq (D, DSQ) -> [128, KD*128]: w_sq_t[p, 128k+j] = W_sq[128k+p, j]
    w_sq_np = np.asarray(W_sq, dtype=np.float32).astype(bf)
    w_sq_t_np = np.ascontiguousarray(
        w_sq_np.reshape(KD, 128, DSQ).transpose(1, 0, 2).reshape(128, D))
    w_ex_t_np = np.ascontiguousarray(np.asarray(W_ex, dtype=np.float32).astype(bf))
    b_sq_np = np.ascontiguousarray(
        np.asarray(b_sq, dtype=np.float32).reshape(DSQ, 1))
    # bexp_full[p, M*k + c] = +b_ex[128k + p]
    bex = np.asarray(b_ex, dtype=np.float32)
    bexp_np = np.ascontiguousarray(
        np.repeat(bex.reshape(KD, 128).T, M, axis=1).reshape(128, SEG))
    idn_np = np.eye(M, dtype=np.float32)

    ei_np = np.zeros((NI * 128, M), dtype=np.float32)
    rows_i = np.arange(M * R)
    ei_np[rows_i, rows_i // R] = 1.0
    ei_t_np = ei_np.reshape(NI, 128, M).transpose(1, 0, 2).reshape(
        128, NI * M).astype(bf)

    in_maps = []
    for j in range(NCORES):
        sl = slice(M * j, M * (j + 1))
        lens_local = lens_i[sl]
        ec2_np = np.zeros((M * T, 2 * M), dtype=np.float32)
        rows = np.arange(M * T)
        cidx = rows // T
        tidx = rows % T
        ec2_np[rows, M + cidx] = 1.0
        keep = tidx < lens_local[cidx]
        ec2_np[rows[keep], cidx[keep]] = 1.0
        ec2_t_np = ec2_np.reshape(NC, 128, 2 * M).transpose(1, 0, 2).reshape(
            128, NC * 2 * M).astype(bf)
        rlens_np = (1.0 / lens_local.astype(np.float64)).astype(
            np.float32).reshape(M, 1)

        in_maps.append({
            "img_rows": np.ascontiguousarray(img_bf[sl].reshape(M * R, D)),
            "cap_rows": np.ascontiguousarray(cap_bf[sl].reshape(M * T, D)),
            "ei_t": np.ascontiguousarray(ei_t_np),
            "ec2_t": np.ascontiguousarray(ec2_t_np),
            "w_sq_t": w_sq_t_np,
            "w_ex_t": w_ex_t_np,
            "b_sq_t": b_sq_np,
            "bexp_full": bexp_np,
            "rlens": rlens_np,
            "idn32": idn_np,
        })
    return in_maps


LAST_RESULT = None
EPS = 1e-8


def kernel(img_embed, cap_embed, lens, W_sq, b_sq, W_ex, b_ex, beta, beta1):
    global LAST_RESULT
    beta_f = float(np.asarray(beta).reshape(-1)[0])
    nc = get_program(beta_f)
    in_maps = make_in_maps(img_embed, cap_embed, lens, W_sq, b_sq, W_ex, b_ex)
    res = run_bass_kernel_spmd(nc, in_maps, core_ids=list(range(NCORES)))
    LAST_RESULT = res
    sims = np.empty((B, B), dtype=np.float32)
    for j in range(NCORES):
        r = res.results[j]
        nvg = r["nvg_out"].astype(np.float64)   # (2M, B)
        num = nvg[0:M]
        vg = nvg[M:2 * M]
        q2 = r["q2_out"].astype(np.float64)
        cv = r["cv_out"].astype(np.float64)     # (M, D)
        rn = 1.0 / (np.sqrt((cv * cv).sum(axis=1, keepdims=True)) + EPS)
        bias = beta_f * cv.sum(axis=1, keepdims=True) * rn
        denom = np.sqrt(q2 + 2.0 * beta_f * vg + beta_f * beta_f * D) + EPS
        simst = (num * rn + bias) / denom       # (M, B) = sims[:, block].T
        sims[:, M * j:M * (j + 1)] = simst.T.astype(np.float32)
    return sims
